# revision 1
# baseline (speedup 1.0000x reference)
"""Multi-head self-attention (B=2, S=4096, D=512, H=8, Dk=64) on 8 TRN2 cores.

Sharding: data-parallel over batch x head-parallel. Core c handles batch
c//4 and head pair (2*(c%4), 2*(c%4)+1). Each core computes Q/K/V
projections for its 128 model dims, full attention for its two heads, and
a partial output projection against its 128 rows of Wo. The host sums the
four partial outputs per batch and adds bo.

On-core layout (bf16 operands, fp32 psum accumulation):
  xT   [d, s]   bf16 via DMA-transpose of host-cast x     (rhs for Q/K, lhsT for V)
  QT/KT [128, S] bf16, head0 in partitions 0:64, head1 in 64:128
  V    [s, 128] bf16, head0 in cols 0:64, head1 in 64:128 (lhsT for ctx)
  scoresT[k, q] fp32 psum from row-packed bf16 matmul pairs (K=64/head)
  attnT = exp(scoresT/8 + mask_bias) bf16, one ACT op per [128, 1024] block
  ctxT [d, q] fp32 psum, col-packed over k blocks; denominators from
  ones-vector matmuls into psum rows 0/32; normalization via fp32 PE
  broadcast of the reciprocals.
"""

import numpy as np
import ml_dtypes
from contextlib import ExitStack

import concourse.bass as bass
import concourse.tile as tile
from concourse import bacc, mybir
from concourse.bass_utils import run_bass_kernel_spmd
from concourse.tile_rust import add_dep_helper

F32 = mybir.dt.float32
F32R = mybir.dt.float32r
BF16 = mybir.dt.bfloat16
EXP = mybir.ActivationFunctionType.Exp

D_MODEL = 512
N_HEADS = 8
D_K = 64
N_CORES = 8
DL = 128          # local model dims per core (2 heads)
Q_BLK = 512       # query block (free dim of scores matmuls)
SCALE = 1.0 / np.sqrt(D_K).item()


def build_kernel(ctx, tc, S, use_mask, use_bq, use_bk, use_bv, d):
    nc = tc.nc
    SB = S // 128    # s blocks of 128
    QB = S // Q_BLK  # query blocks of 512
    KB = S // 128    # key blocks of 128

    sp = ctx.enter_context(tc.tile_pool(name="sp", bufs=1))
    psum = ctx.enter_context(tc.tile_pool(name="psum", bufs=1, space="PSUM"))
    # psum budget (8 banks): scores 2x[128,1024]=4, ctx 2x[128,512]=2,
    # den 2x[<=1 bank]=2. All other matmul outputs share the ctx/den tags.

    # ---- constants ----
    ones_f = sp.tile([128, 1], F32, tag="ones_f")
    nc.vector.memset(ones_f, 1.0)
    ones_col = sp.tile([128, 1], BF16, tag="ones_col")  # lhsT of denominator mms
    nc.vector.tensor_copy(ones_col, ones_f)
    ones_rep_f = sp.tile([33, 128], F32, tag="ones_rep_f")
    nc.vector.memset(ones_rep_f, 1.0)
    ones_rep = sp.tile([33, 128], F32R, tag="ones_rep")  # lhsT of broadcast mms
    nc.vector.tensor_copy(ones_rep, ones_rep_f)

    # ---- phase 1: DMA-transpose x (bf16) into xT [128, 4, S]. The xbar
    # transposes and regular DMAs must not overlap (S2M xbar-mode
    # transition hazard): transposes run first, all other DMAs wait. ----
    xt = sp.tile([128, 4, S], BF16, tag="xt")
    t_insts = []
    for t in range(4):
        t_insts.append(nc.sync.dma_start_transpose(
            xt[:, t, :], d["xb"].ap()[:, t * 128:(t + 1) * 128]))

    def wdma(out_ap, in_ap):
        ins = nc.sync.dma_start(out_ap, in_ap)
        for t_inst in t_insts:
            add_dep_helper(ins.ins, t_inst.ins, reason="xbar-mode serialize")
        return ins

    wq_sb = sp.tile([128, 4, 128], BF16, tag="wq")
    wdma(wq_sb, d["wq"].ap().rearrange("(t p) d -> p t d", p=128))
    wk_sb = sp.tile([128, 4, 128], BF16, tag="wk")
    wdma(wk_sb, d["wk"].ap().rearrange("(t p) d -> p t d", p=128))
    wv_sb = sp.tile([128, 4, 128], BF16, tag="wv")
    wdma(wv_sb, d["wv"].ap().rearrange("(t p) d -> p t d", p=128))
    wo_sb = sp.tile([128, 512], F32R, tag="wo")
    wdma(wo_sb, d["wo"].ap())
    if use_bq:
        bq_sb = sp.tile([128, 1], F32, tag="bq")
        wdma(bq_sb, d["bq"].ap()[:, None])
    if use_bk:
        bk_sb = sp.tile([128, 1], F32, tag="bk")
        wdma(bk_sb, d["bk"].ap()[:, None])
    if use_bv:
        bv_sb = sp.tile([1, 128], F32, tag="bv")
        wdma(bv_sb, d["bv"].ap()[None, :])
        ones_row = sp.tile([1, 128], F32, tag="ones_row")
        nc.vector.memset(ones_row, 1.0)
    if use_mask:
        mb_sb = sp.tile([128, KB], F32, tag="mb")
        wdma(mb_sb, d["mb"].ap())

    # ---- PE warm-up: the HAM clock gate needs ~3.4us of sustained matmul
    # activity to lift the PE from 1.2 to 2.4 GHz; run throwaway matmuls
    # while the transposes stream so phase 2 starts at full clock. ----
    scratch = sp.tile([128, 512], BF16, tag="scratch")
    nc.vector.memset(scratch, 0.0)
    for _ in range(48):
        pw = psum.tile([33, 512], F32, tag="den", bufs=2, name="pw")
        nc.tensor.matmul(pw[0:1, :], scratch[:, 0:1], scratch)


    # ---- phase 2: projections ----
    qt = sp.tile([128, S], BF16, tag="qt")
    kt = sp.tile([128, S], BF16, tag="kt")
    for dst, w_sb, b_sb in (
        (qt, wq_sb, bq_sb if use_bq else None),
        (kt, wk_sb, bk_sb if use_bk else None),
    ):
        for sc in range(S // 512):
            pp = psum.tile([128, 512], F32, tag="ctx", bufs=2)
            for t in range(4):
                nc.tensor.matmul(
                    pp, w_sb[:, t, :], xt[:, t, sc * 512:(sc + 1) * 512],
                    start=(t == 0), stop=(t == 3))
            out = dst[:, sc * 512:(sc + 1) * 512]
            if b_sb is not None:
                nc.vector.tensor_scalar_add(out, pp, b_sb[:, 0:1])
            else:
                nc.vector.tensor_copy(out, pp)

    v_all = sp.tile([128, SB, 128], BF16, tag="v")
    for sb in range(SB):
        pv = psum.tile([128, 128], F32, tag="den", bufs=2)
        for t in range(4):
            nc.tensor.matmul(
                pv, xt[:, t, sb * 128:(sb + 1) * 128], wv_sb[:, t, :],
                start=(t == 0), stop=(t == 3 and not use_bv))
        if use_bv:
            nc.tensor.matmul(pv, ones_row[0:1, :], bv_sb[0:1, :],
                             start=False, stop=True)
        nc.vector.tensor_copy(v_all[:, sb, :], pv)

    # ---- phase 3: attention ----
    ctxn = sp.tile([128, S], F32R, tag="ctxn")
    pending_tail = None
    for qb in range(QB):
        qs = slice(qb * Q_BLK, (qb + 1) * Q_BLK)
        pc = psum.tile([128, 512], F32, tag="ctx", bufs=2)
        pd = psum.tile([33, 512], F32, tag="den", bufs=2)

        def scores_block(kb):
            # one query-block column of scores for both heads + its exp
            ks = slice(kb * 128, (kb + 1) * 128)
            ps = psum.tile([128, 1024], F32, tag="scores", bufs=2, name="ps")
            nc.tensor.matmul(ps[:, 0:512], kt[0:64, ks], qt[0:64, qs])
            nc.tensor.matmul(ps[:, 512:1024], kt[64:128, ks], qt[64:128, qs])
            attn = sp.tile([128, 1024], BF16, tag="attn", bufs=3, name="attn")
            nc.scalar.activation(
                attn, ps, EXP, scale=SCALE,
                bias=mb_sb[:, kb:kb + 1] if use_mask else 0.0)
            return attn

        # Software-pipelined: scores/exp for kb+1 are emitted before the
        # ctx/den matmuls of kb, so the PE streams scores(kb+1) while the
        # ACT engine computes exp(kb) — the serial exp->ctx->scores->exp
        # chain would otherwise set the loop period.
        attn = scores_block(0)
        anchor = None
        for kb in range(KB):
            attn_next = scores_block(kb + 1) if kb + 1 < KB else None
            if kb == min(8, KB - 1) and pending_tail is not None:
                pending_tail(anchor)
                pending_tail = None
            first, last = kb == 0, kb == KB - 1
            m = nc.tensor.matmul(pc[0:64, :], v_all[:, kb, 0:64],
                                 attn[:, 0:512], start=first, stop=last,
                                 skip_group_check=True)
            if kb == min(7, KB - 2):
                anchor = m
            nc.tensor.matmul(pc[64:128, :], v_all[:, kb, 64:128],
                             attn[:, 512:1024], start=first, stop=last,
                             skip_group_check=True)
            nc.tensor.matmul(pd[0:1, :], ones_col[:, 0:1],
                             attn[:, 0:512], start=first, stop=last,
                             skip_group_check=True)
            nc.tensor.matmul(pd[32:33, :], ones_col[:, 0:1],
                             attn[:, 512:1024], start=first, stop=last,
                             skip_group_check=True)
            attn = attn_next

        # Denominator extraction + reciprocals start immediately (DVE is
        # idle during the matmul loop)...
        den_sb = sp.tile([33, 512], F32, tag="den_sb", bufs=2)
        nc.vector.tensor_copy(den_sb[0:1, :], pd[0:1, :])
        nc.vector.tensor_copy(den_sb[32:33, :], pd[32:33, :])
        rcp = sp.tile([33, 512], F32R, tag="rcp", bufs=2)
        with nc.allow_low_precision(reason="f32r-rounded reciprocal feeds the fp32r broadcast matmul"):
            nc.vector.reciprocal(rcp[0:1, :], den_sb[0:1, :])
            nc.vector.reciprocal(rcp[32:33, :], den_sb[32:33, :])

        # ...but the PE part of the tail (broadcast matmuls + output
        # projection) is deferred by one query block, so the PE never
        # stalls on the reciprocal chain — that stall re-throttles the
        # HAM clock to K=4/8 and halves matmul throughput.
        def tail(anchor, qb=qb, qs=qs, pc=pc, rcp=rcp):
            pr0 = psum.tile([128, 512], F32, tag="den", bufs=2, name="pr0")
            m0 = nc.tensor.matmul(pr0, ones_rep[0:1, :], rcp[0:1, :])
            rep0 = sp.tile([128, 512], F32, tag="rep", bufs=2, name="rep0")
            nc.vector.tensor_copy(rep0, pr0)
            pr1 = psum.tile([128, 512], F32, tag="den", bufs=2, name="pr1")
            m1 = nc.tensor.matmul(pr1, ones_rep[32:33, :], rcp[32:33, :])
            rep1 = sp.tile([128, 512], F32, tag="rep", bufs=2, name="rep1")
            nc.vector.tensor_copy(rep1, pr1)
            if anchor is not None:
                add_dep_helper(m0.ins, anchor.ins, reason="defer tail mm")
                add_dep_helper(m1.ins, anchor.ins, reason="defer tail mm")
            nc.vector.tensor_mul(ctxn[0:64, qs], pc[0:64, :], rep0[0:64, :])
            nc.vector.tensor_mul(ctxn[64:128, qs], pc[64:128, :], rep1[64:128, :])
            for i in range(Q_BLK // 128):
                sb = qb * (Q_BLK // 128) + i
                po = psum.tile([128, 512], F32, tag="ctx", bufs=2, name="po")
                nc.tensor.matmul(po, ctxn[:, sb * 128:(sb + 1) * 128], wo_sb)
                ob = sp.tile([128, 512], F32, tag="ob", bufs=3, name="ob")
                nc.vector.tensor_copy(ob, po)
                nc.sync.dma_start(d["out"].ap()[sb * 128:(sb + 1) * 128, :], ob)

        pending_tail = tail

    pending_tail(None)


def build_program(S=4096, use_mask=False, use_bq=False, use_bk=False,
                  use_bv=False, enable_asserts=False):
    nc = bacc.Bacc("TRN2", target_bir_lowering=False, debug=False,
                   enable_asserts=enable_asserts, num_devices=N_CORES,
                   name="mha")
    d = {
        "xb": nc.dram_tensor("xb", [S, D_MODEL], BF16, kind="ExternalInput"),
        "wq": nc.dram_tensor("wq", [D_MODEL, DL], BF16, kind="ExternalInput"),
        "wk": nc.dram_tensor("wk", [D_MODEL, DL], BF16, kind="ExternalInput"),
        "wv": nc.dram_tensor("wv", [D_MODEL, DL], BF16, kind="ExternalInput"),
        "wo": nc.dram_tensor("wo", [DL, D_MODEL], F32R, kind="ExternalInput"),
        "out": nc.dram_tensor("out", [S, D_MODEL], F32, kind="ExternalOutput"),
    }
    if use_bq:
        d["bq"] = nc.dram_tensor("bq", [DL], F32, kind="ExternalInput")
    if use_bk:
        d["bk"] = nc.dram_tensor("bk", [DL], F32, kind="ExternalInput")
    if use_bv:
        d["bv"] = nc.dram_tensor("bv", [DL], F32, kind="ExternalInput")
    if use_mask:
        d["mb"] = nc.dram_tensor("mb", [128, S // 128], F32,
                                 kind="ExternalInput")
    with tile.TileContext(nc) as tc:
        with ExitStack() as ctx:
            build_kernel(ctx, tc, S, use_mask, use_bq, use_bk, use_bv, d)
    nc.compile()
    return nc


_cache = {}


def _program(key):
    if key not in _cache:
        _cache[key] = build_program(
            S=4096, use_mask=key[0], use_bq=key[1], use_bk=key[2],
            use_bv=key[3])
    return _cache[key]


def kernel(x, mask, Wq, bq, Wk, bk, Wv, bv, Wo, bo, _results_hook=None):
    x = np.asarray(x, np.float32)
    mask = np.asarray(mask)
    B, S, _ = x.shape
    use_mask = bool((mask == 0).any())
    use_bq = bool(np.asarray(bq).any())
    use_bk = bool(np.asarray(bk).any())
    use_bv = bool(np.asarray(bv).any())
    nc = _program((use_mask, use_bq, use_bk, use_bv))

    in_maps = []
    for c in range(N_CORES):
        b, j = divmod(c, N_CORES // B)
        ds = slice(j * DL, (j + 1) * DL)
        m = {
            "xb": np.ascontiguousarray(x[b]).astype(ml_dtypes.bfloat16),
            "wq": np.ascontiguousarray(Wq[:, ds]).astype(ml_dtypes.bfloat16),
            "wk": np.ascontiguousarray(Wk[:, ds]).astype(ml_dtypes.bfloat16),
            "wv": np.ascontiguousarray(Wv[:, ds]).astype(ml_dtypes.bfloat16),
            "wo": np.ascontiguousarray(Wo[ds, :], dtype=np.float32),
        }
        if use_bq:
            m["bq"] = np.ascontiguousarray(bq[ds], dtype=np.float32)
        if use_bk:
            m["bk"] = np.ascontiguousarray(bk[ds], dtype=np.float32)
        if use_bv:
            m["bv"] = np.ascontiguousarray(bv[ds], dtype=np.float32)
        if use_mask:
            mb = np.where(np.asarray(mask[b]) == 0, -1e9, 0.0).astype(np.float32)
            m["mb"] = np.ascontiguousarray(mb.reshape(S // 128, 128).T)
        in_maps.append(m)

    res = run_bass_kernel_spmd(nc, in_maps, core_ids=list(range(N_CORES)))
    if _results_hook is not None:
        _results_hook(res)
    out = np.zeros((B, S, D_MODEL), np.float32)
    for c in range(N_CORES):
        b = c // (N_CORES // B)
        out[b] += res.results[c]["out"]
    out += np.asarray(bo, np.float32)
    return out



# revision 7
# speedup vs baseline: 1.1498x; 1.1498x over previous
"""Multi-head self-attention (B=2, S=4096, D=512, H=8, Dk=64) on 8 TRN2 cores.

Sharding: data-parallel over batch x head-parallel. Core c handles batch
c//4 and head pair (2*(c%4), 2*(c%4)+1). Each core computes Q/K/V
projections for its 128 model dims, full attention for its two heads, and
a partial output projection against its 128 rows of Wo. The host sums the
four partial outputs per batch and adds bo.

The kernel is paced by the ACT engine (exp of 2*S^2 = 33.5M scores per
core at 1 elem/lane/cycle @ 1.2 GHz ~= 284us); everything else is
structured to keep ACT streaming back-to-back:
  - x arrives host-transposed (xT [512, S] bf16) and is DMA'd in S-chunks;
    Q/K/V projections run per-chunk so the first exp issues ~8us in
    (no xbar DMA transposes, no serial 48us prologue).
  - softmax denominator rides as a 65th column of V (ones), so the ctx
    matmul pair computes ctx+den with no separate den matmuls.
  - den rows are PE-transposed to [q-partition, head] form; reciprocal and
    the output normalization are per-partition DVE ops (no fp32r broadcast
    matmuls, no single-lane 512-wide reciprocals).
  - output projection runs per head (K=64 row-packed concurrent pair) on
    unnormalized ctx in bf16; DVE applies the two reciprocals and sums.
"""

import numpy as np
import ml_dtypes
from contextlib import ExitStack

import concourse.bass as bass
import concourse.tile as tile
from concourse import bacc, mybir
from concourse.bass_utils import run_bass_kernel_spmd

F32 = mybir.dt.float32
BF16 = mybir.dt.bfloat16
EXP = mybir.ActivationFunctionType.Exp
MULT = mybir.AluOpType.mult
ADD = mybir.AluOpType.add

D_MODEL = 512
N_HEADS = 8
D_K = 64
N_CORES = 8
DL = 128          # local model dims per core (2 heads)
Q_BLK = 512       # query block (free dim of scores matmuls)
SCALE = 1.0 / np.sqrt(D_K).item()


def build_kernel(ctx, tc, S, use_mask, use_bq, use_bk, use_bv, d):
    nc = tc.nc
    SB = S // 128    # 128-wide s blocks
    QB = S // Q_BLK  # query blocks
    KB = SB          # key blocks of 128
    NCH = S // 512   # s-chunks for the load/projection pipeline

    sp = ctx.enter_context(tc.tile_pool(name="sp", bufs=1))
    psum = ctx.enter_context(tc.tile_pool(name="psum", bufs=1, space="PSUM"))
    # psum budget (8 banks): scores 2x[128,1024]=4, pc_h0/pc_h1 [65,512]=2,
    # tail/proj rotating tag = 2.

    # ---- constants ----
    # selector: den_row[0]=den_h0, den_row[32]=den_h1; tr = den_row_chunk^T
    # @ sel puts den_h0 in tr col 0 and den_h1 in tr col 1.
    sel = sp.tile([33, 2], F32, tag="sel")
    nc.vector.memset(sel, 0.0)
    nc.vector.memset(sel[0:1, 0:1], 1.0)
    nc.vector.memset(sel[32:33, 1:2], 1.0)

    # ---- weight DMAs (no DMA transposes anywhere: no xbar hazard) ----
    wq_sb = sp.tile([128, 4, 128], BF16, tag="wq")
    nc.sync.dma_start(wq_sb, d["wq"].ap().rearrange("(t p) d -> p t d", p=128))
    wk_sb = sp.tile([128, 4, 128], BF16, tag="wk")
    nc.sync.dma_start(wk_sb, d["wk"].ap().rearrange("(t p) d -> p t d", p=128))
    wv_sb = sp.tile([128, 4, 128], BF16, tag="wv")
    nc.sync.dma_start(wv_sb, d["wv"].ap().rearrange("(t p) d -> p t d", p=128))
    wo_sb = sp.tile([128, 512], BF16, tag="wo")
    nc.sync.dma_start(wo_sb, d["wo"].ap())
    if use_bq:
        bq_sb = sp.tile([128, 1], F32, tag="bq")
        nc.sync.dma_start(bq_sb, d["bq"].ap()[:, None])
    if use_bk:
        bk_sb = sp.tile([128, 1], F32, tag="bk")
        nc.sync.dma_start(bk_sb, d["bk"].ap()[:, None])
    if use_bv:
        bv_sb = sp.tile([1, 128], F32, tag="bv")
        nc.sync.dma_start(bv_sb, d["bv"].ap()[None, :])
        ones_row = sp.tile([1, 128], F32, tag="ones_row")
        nc.vector.memset(ones_row, 1.0)
    if use_mask:
        mb_sb = sp.tile([128, KB], F32, tag="mb")
        nc.sync.dma_start(mb_sb, d["mb"].ap())

    # ---- x chunks + projections, pipelined ----
    xt = sp.tile([128, 4, S], BF16, tag="xt")
    for c in range(NCH):
        cs = slice(c * 512, (c + 1) * 512)
        nc.sync.dma_start(
            xt[:, :, cs],
            d["xb"].ap().rearrange("(t p) s -> p t s", p=128)[:, :, cs])

    qt = sp.tile([128, S], BF16, tag="qt")
    kt = sp.tile([128, S], BF16, tag="kt")
    # v_ext: per key block, cols 0:64 = V head0, 64 = ones, 65:129 = V
    # head1, 129 = ones -> the ctx matmul's 65th output row is the softmax
    # denominator.
    v_ext = sp.tile([128, SB, 130], BF16, tag="v")
    nc.vector.memset(v_ext[:, :, 64:65], 1.0)
    nc.vector.memset(v_ext[:, :, 129:130], 1.0)

    def project_chunk(c):
        cs = slice(c * 512, (c + 1) * 512)
        for dst, w_sb, b_sb in (
            (qt, wq_sb, bq_sb if use_bq else None),
            (kt, wk_sb, bk_sb if use_bk else None),
        ):
            pp = psum.tile([128, 512], F32, tag="tail", bufs=2, name="pp")
            for t in range(4):
                nc.tensor.matmul(pp, w_sb[:, t, :], xt[:, t, cs],
                                 start=(t == 0), stop=(t == 3))
            if b_sb is not None:
                nc.vector.tensor_scalar_add(dst[:, cs], pp, b_sb[:, 0:1])
            else:
                nc.vector.tensor_copy(dst[:, cs], pp)
        for i in range(4):
            sb = 4 * c + i
            pv = psum.tile([128, 128], F32, tag="tail", bufs=2, name="pv")
            for t in range(4):
                nc.tensor.matmul(
                    pv, xt[:, t, sb * 128:(sb + 1) * 128], wv_sb[:, t, :],
                    start=(t == 0), stop=(t == 3 and not use_bv))
            if use_bv:
                nc.tensor.matmul(pv, ones_row[0:1, :], bv_sb[0:1, :],
                                 start=False, stop=True)
            nc.vector.tensor_copy(v_ext[:, sb, 0:64], pv[:, 0:64])
            nc.vector.tensor_copy(v_ext[:, sb, 65:129], pv[:, 64:128])

    for c in range(NCH):
        project_chunk(c)

    # ---- attention ----
    pending_tail = None
    for qb in range(QB):
        qs = slice(qb * Q_BLK, (qb + 1) * Q_BLK)
        pc0 = psum.tile([65, 512], F32, tag="pc0", bufs=1)
        pc1 = psum.tile([65, 512], F32, tag="pc1", bufs=1)

        def scores_block(kb):
            ks = slice(kb * 128, (kb + 1) * 128)
            ps = psum.tile([128, 1024], F32, tag="ps", bufs=2, name="ps")
            nc.tensor.matmul(ps[:, 0:512], kt[0:64, ks], qt[0:64, qs])
            nc.tensor.matmul(ps[:, 512:1024], kt[64:128, ks], qt[64:128, qs])
            attn = sp.tile([128, 1024], BF16, tag="attn", bufs=3, name="attn")
            nc.scalar.activation(
                attn, ps, EXP, scale=SCALE,
                bias=mb_sb[:, kb:kb + 1] if use_mask else 0.0)
            return attn

        # Software-pipelined: scores/exp for kb+1 are emitted before the
        # ctx matmuls of kb so the PE streams scores(kb+1) while ACT
        # computes exp(kb). The previous qb's tail work is spread across
        # the first iterations so ACT never waits on tail-induced PE work.
        attn = scores_block(0)
        for kb in range(KB):
            attn_next = scores_block(kb + 1) if kb + 1 < KB else None
            # spread the previous qb's tail stages (PE transposes, output
            # projections) one per two kb iterations
            if pending_tail and kb % 2 == 1:
                pending_tail.pop(0)()
            first, last = kb == 0, kb == KB - 1
            nc.tensor.matmul(pc0, v_ext[:, kb, 0:65], attn[:, 0:512],
                             start=first, stop=last, skip_group_check=True)
            nc.tensor.matmul(pc1, v_ext[:, kb, 65:130], attn[:, 512:1024],
                             start=first, stop=last, skip_group_check=True)
            attn = attn_next
        while pending_tail:
            pending_tail.pop(0)()

        # ---- tail for this qb ----
        # The PSUM extraction (DVE) is emitted NOW so its WAR edges land
        # before the next qb's first ctx matmul reuses pc0/pc1 (bufs=1).
        den_row = sp.tile([33, 512], F32, tag="den_row", bufs=2,
                          name="den_row")
        nc.vector.tensor_copy(den_row[0:1, :], pc0[64:65, :])
        nc.vector.tensor_copy(den_row[32:33, :], pc1[64:65, :])
        ctxn = sp.tile([128, 512], BF16, tag="ctxn", bufs=2, name="ctxn")
        nc.vector.tensor_copy(ctxn[0:64, :], pc0[0:64, :])
        nc.vector.tensor_copy(ctxn[64:128, :], pc1[0:64, :])

        def make_stages(qb=qb, den_row=den_row, ctxn=ctxn):
            state = {}

            def stage_rcp(i):
                # den -> [q-partition, head] via a tiny PE matmul against
                # the selector; rcp then runs on all 128 lanes
                tr = psum.tile([128, 2], F32, tag="tail", bufs=2, name="tr")
                nc.tensor.matmul(tr, den_row[:, i * 128:(i + 1) * 128], sel)
                rcp = sp.tile([128, 2], F32, tag="rcp", bufs=4, name="rcp")
                nc.vector.reciprocal(rcp, tr)
                state[i] = rcp

            def stage_proj(i):
                rcp = state[i]
                cch = slice(i * 128, (i + 1) * 128)
                po0 = psum.tile([128, 512], F32, tag="tail", bufs=2,
                                name="po0")
                nc.tensor.matmul(po0, ctxn[0:64, cch], wo_sb[0:64, :])
                po1 = psum.tile([128, 512], F32, tag="tail", bufs=2,
                                name="po1")
                nc.tensor.matmul(po1, ctxn[64:128, cch], wo_sb[64:128, :])
                tmp = sp.tile([128, 512], F32, tag="tmp", bufs=2, name="tmp")
                nc.vector.tensor_scalar_mul(tmp, po1, rcp[:, 1:2])
                ob = sp.tile([128, 512], F32, tag="ob", bufs=3, name="ob")
                nc.vector.scalar_tensor_tensor(
                    ob, po0, rcp[:, 0:1], tmp, MULT, ADD)
                sb = qb * 4 + i
                nc.sync.dma_start(
                    d["out"].ap()[sb * 128:(sb + 1) * 128, :], ob)

            stages = []
            for i in range(4):
                stages.append(lambda i=i: stage_rcp(i))
            for i in range(4):
                stages.append(lambda i=i: stage_proj(i))
            return stages

        pending_tail = make_stages()

    while pending_tail:
        pending_tail.pop(0)()


def build_program(S=4096, use_mask=False, use_bq=False, use_bk=False,
                  use_bv=False, enable_asserts=False):
    nc = bacc.Bacc("TRN2", target_bir_lowering=False, debug=False,
                   enable_asserts=enable_asserts, num_devices=N_CORES,
                   name="mha")
    d = {
        "xb": nc.dram_tensor("xb", [D_MODEL, S], BF16, kind="ExternalInput"),
        "wq": nc.dram_tensor("wq", [D_MODEL, DL], BF16, kind="ExternalInput"),
        "wk": nc.dram_tensor("wk", [D_MODEL, DL], BF16, kind="ExternalInput"),
        "wv": nc.dram_tensor("wv", [D_MODEL, DL], BF16, kind="ExternalInput"),
        "wo": nc.dram_tensor("wo", [DL, D_MODEL], BF16, kind="ExternalInput"),
        "out": nc.dram_tensor("out", [S, D_MODEL], F32, kind="ExternalOutput"),
    }
    if use_bq:
        d["bq"] = nc.dram_tensor("bq", [DL], F32, kind="ExternalInput")
    if use_bk:
        d["bk"] = nc.dram_tensor("bk", [DL], F32, kind="ExternalInput")
    if use_bv:
        d["bv"] = nc.dram_tensor("bv", [DL], F32, kind="ExternalInput")
    if use_mask:
        d["mb"] = nc.dram_tensor("mb", [128, S // 128], F32,
                                 kind="ExternalInput")
    with tile.TileContext(nc) as tc:
        with ExitStack() as ctx:
            build_kernel(ctx, tc, S, use_mask, use_bq, use_bk, use_bv, d)
    nc.compile()
    return nc


_cache = {}


def _program(key):
    if key not in _cache:
        _cache[key] = build_program(
            S=4096, use_mask=key[0], use_bq=key[1], use_bk=key[2],
            use_bv=key[3])
    return _cache[key]


def kernel(x, mask, Wq, bq, Wk, bk, Wv, bv, Wo, bo, _results_hook=None):
    x = np.asarray(x, np.float32)
    mask = np.asarray(mask)
    B, S, _ = x.shape
    use_mask = bool((mask == 0).any())
    use_bq = bool(np.asarray(bq).any())
    use_bk = bool(np.asarray(bk).any())
    use_bv = bool(np.asarray(bv).any())
    nc = _program((use_mask, use_bq, use_bk, use_bv))

    in_maps = []
    for c in range(N_CORES):
        b, j = divmod(c, N_CORES // B)
        ds = slice(j * DL, (j + 1) * DL)
        m = {
            "xb": np.ascontiguousarray(x[b].T).astype(ml_dtypes.bfloat16),
            "wq": np.ascontiguousarray(Wq[:, ds]).astype(ml_dtypes.bfloat16),
            "wk": np.ascontiguousarray(Wk[:, ds]).astype(ml_dtypes.bfloat16),
            "wv": np.ascontiguousarray(Wv[:, ds]).astype(ml_dtypes.bfloat16),
            "wo": np.ascontiguousarray(Wo[ds, :]).astype(ml_dtypes.bfloat16),
        }
        if use_bq:
            m["bq"] = np.ascontiguousarray(bq[ds], dtype=np.float32)
        if use_bk:
            m["bk"] = np.ascontiguousarray(bk[ds], dtype=np.float32)
        if use_bv:
            m["bv"] = np.ascontiguousarray(bv[ds], dtype=np.float32)
        if use_mask:
            mb = np.where(np.asarray(mask[b]) == 0, -1e9, 0.0).astype(np.float32)
            m["mb"] = np.ascontiguousarray(mb.reshape(S // 128, 128).T)
        in_maps.append(m)

    res = run_bass_kernel_spmd(nc, in_maps, core_ids=list(range(N_CORES)))
    if _results_hook is not None:
        _results_hook(res)
    out = np.zeros((B, S, D_MODEL), np.float32)
    for c in range(N_CORES):
        b = c // (N_CORES // B)
        out[b] += res.results[c]["out"]
    out += np.asarray(bo, np.float32)
    return out


# revision 14
# speedup vs baseline: 1.1870x; 1.0324x over previous
"""Multi-head self-attention (B=2, S=4096, D=512, H=8, Dk=64) on 8 TRN2 cores.

Sharding: data-parallel over batch x head-parallel. Core c handles batch
c//4 and head pair (2*(c%4), 2*(c%4)+1). Each core computes Q/K/V
projections for its 128 model dims, full attention for its two heads, and
a partial output projection against its 128 rows of Wo. The host sums the
four partial outputs per batch and adds bo.

The kernel is paced by the ACT engine (exp of 2*S^2 = 33.5M scores per
core at 1 elem/lane/cycle @ 1.2 GHz ~= 284us); everything else is
structured to keep ACT streaming back-to-back:
  - x arrives host-transposed (xT [512, S] bf16) and is DMA'd in S-chunks;
    Q/K/V projections run per-chunk so the first exp issues ~8us in
    (no xbar DMA transposes, no serial 48us prologue).
  - softmax denominator rides as a 65th column of V (ones), so the ctx
    matmul pair computes ctx+den with no separate den matmuls.
  - den rows are PE-transposed to [q-partition, head] form; reciprocal and
    the output normalization are per-partition DVE ops (no fp32r broadcast
    matmuls, no single-lane 512-wide reciprocals).
  - output projection runs per head (K=64 row-packed concurrent pair) on
    unnormalized ctx in bf16; DVE applies the two reciprocals and sums.
"""

import numpy as np
import ml_dtypes
from contextlib import ExitStack

import concourse.bass as bass
import concourse.tile as tile
from concourse import bacc, mybir
from concourse.bass_utils import run_bass_kernel_spmd

F32 = mybir.dt.float32
F16 = mybir.dt.float16
BF16 = mybir.dt.bfloat16
EXP = mybir.ActivationFunctionType.Exp
MULT = mybir.AluOpType.mult
ADD = mybir.AluOpType.add

D_MODEL = 512
N_HEADS = 8
D_K = 64
N_CORES = 8
DL = 128          # local model dims per core (2 heads)
Q_BLK = 512       # query block (free dim of scores matmuls)
SCALE = 1.0 / np.sqrt(D_K).item()


def build_kernel(ctx, tc, S, use_mask, use_bq, use_bk, use_bv, d):
    nc = tc.nc
    SB = S // 128    # 128-wide s blocks
    QB = S // Q_BLK  # query blocks
    KB = SB          # key blocks of 128
    NCH = S // 512   # s-chunks for the load/projection pipeline

    sp = ctx.enter_context(tc.tile_pool(name="sp", bufs=1))
    psum = ctx.enter_context(tc.tile_pool(name="psum", bufs=1, space="PSUM"))
    # psum budget (8 banks): scores 2x[128,1024]=4, pc_h0/pc_h1 [65,512]=2,
    # tail/proj rotating tag = 2.

    # ---- constants ----
    # selector: den_row[0]=den_h0, den_row[32]=den_h1; tr = den_row_chunk^T
    # @ sel puts den_h0 in tr col 0 and den_h1 in tr col 1. fp16 keeps the
    # matmul single-pass (fp32 matmuls run LOW/HIGH double passes) at
    # ~5e-4 relative rounding on the denominator.
    sel = sp.tile([33, 2], F16, tag="sel")
    nc.vector.memset(sel, 0.0)
    nc.vector.memset(sel[0:1, 0:1], 1.0)
    nc.vector.memset(sel[32:33, 1:2], 1.0)

    # ---- weight DMAs (no DMA transposes anywhere: no xbar hazard) ----
    wq_sb = sp.tile([128, 4, 128], BF16, tag="wq")
    nc.sync.dma_start(wq_sb, d["wq"].ap().rearrange("(t p) d -> p t d", p=128))
    wk_sb = sp.tile([128, 4, 128], BF16, tag="wk")
    nc.sync.dma_start(wk_sb, d["wk"].ap().rearrange("(t p) d -> p t d", p=128))
    wv_sb = sp.tile([128, 4, 128], BF16, tag="wv")
    nc.sync.dma_start(wv_sb, d["wv"].ap().rearrange("(t p) d -> p t d", p=128))
    wo_sb = sp.tile([128, 512], BF16, tag="wo")
    nc.sync.dma_start(wo_sb, d["wo"].ap())
    if use_bq:
        bq_sb = sp.tile([128, 1], F32, tag="bq")
        nc.sync.dma_start(bq_sb, d["bq"].ap()[:, None])
    if use_bk:
        bk_sb = sp.tile([128, 1], F32, tag="bk")
        nc.sync.dma_start(bk_sb, d["bk"].ap()[:, None])
    if use_bv:
        bv_sb = sp.tile([1, 128], F32, tag="bv")
        nc.sync.dma_start(bv_sb, d["bv"].ap()[None, :])
        ones_row = sp.tile([1, 128], F32, tag="ones_row")
        nc.vector.memset(ones_row, 1.0)
    if use_mask:
        mb_sb = sp.tile([128, KB], F32, tag="mb")
        nc.sync.dma_start(mb_sb, d["mb"].ap())

    # ---- x chunks + projections, pipelined ----
    xt = sp.tile([128, 4, S], BF16, tag="xt")
    for c in range(NCH):
        cs = slice(c * 512, (c + 1) * 512)
        nc.sync.dma_start(
            xt[:, :, cs],
            d["xb"].ap().rearrange("(t p) s -> p t s", p=128)[:, :, cs])

    qt = sp.tile([128, S], BF16, tag="qt")
    kt = sp.tile([128, S], BF16, tag="kt")
    # v_ext: per key block, cols 0:64 = V head0, 64 = ones, 65:129 = V
    # head1, 129 = ones -> the ctx matmul's 65th output row is the softmax
    # denominator.
    v_ext = sp.tile([128, SB, 130], BF16, tag="v")
    nc.vector.memset(v_ext[:, :, 64:65], 1.0)
    nc.vector.memset(v_ext[:, :, 129:130], 1.0)

    def project_qk(c, dst, w_sb, b_sb):
        cs = slice(c * 512, (c + 1) * 512)
        pp = psum.tile([128, 512], F32, tag="tail", bufs=2, name="pp")
        for t in range(4):
            nc.tensor.matmul(pp, w_sb[:, t, :], xt[:, t, cs],
                             start=(t == 0), stop=(t == 3))
        if b_sb is not None:
            nc.vector.tensor_scalar_add(dst[:, cs], pp, b_sb[:, 0:1])
        else:
            nc.vector.tensor_copy(dst[:, cs], pp)

    def project_v(sb):
        pv = psum.tile([128, 128], F32, tag="tail", bufs=2, name="pv")
        for t in range(4):
            nc.tensor.matmul(
                pv, xt[:, t, sb * 128:(sb + 1) * 128], wv_sb[:, t, :],
                start=(t == 0), stop=(t == 3 and not use_bv))
        if use_bv:
            nc.tensor.matmul(pv, ones_row[0:1, :], bv_sb[0:1, :],
                             start=False, stop=True)
        nc.vector.tensor_copy(v_ext[:, sb, 0:64], pv[:, 0:64])
        nc.vector.tensor_copy(v_ext[:, sb, 65:129], pv[:, 64:128])

    def project_q(c):
        project_qk(c, qt, wq_sb, bq_sb if use_bq else None)

    def project_k(c):
        project_qk(c, kt, wk_sb, bk_sb if use_bk else None)

    # prologue: chunk 0 only — attention starts as soon as it lands.
    # kt/v chunks 1..7 are emitted just-in-time inside qb0's kb loop; qt
    # chunk c is deferred into qb c-1's loop (qb0 only reads qt chunk 0).
    project_q(0)
    project_k(0)
    for i in range(4):
        project_v(i)

    # ---- attention ----
    def scores_block(qb, kb):
        qs = slice(qb * Q_BLK, (qb + 1) * Q_BLK)
        ks = slice(kb * 128, (kb + 1) * 128)
        ps = psum.tile([128, 1024], F32, tag="ps", bufs=2, name="ps")
        nc.tensor.matmul(ps[:, 0:512], kt[0:64, ks], qt[0:64, qs])
        nc.tensor.matmul(ps[:, 512:1024], kt[64:128, ks], qt[64:128, qs])
        attn = sp.tile([128, 1024], BF16, tag="attn", bufs=4, name="attn")
        nc.scalar.activation(
            attn, ps, EXP, scale=SCALE,
            bias=mb_sb[:, kb:kb + 1] if use_mask else 0.0)
        return attn

    pending_tail = []
    attn = scores_block(0, 0)
    for qb in range(QB):
        pc0 = psum.tile([65, 512], F32, tag="pc0", bufs=1)
        pc1 = psum.tile([65, 512], F32, tag="pc1", bufs=1)

        # Software-pipelined: scores/exp for the next (qb, kb) — crossing
        # qb boundaries — are emitted before the ctx matmuls of kb, so the
        # PE streams the next scores while ACT computes the current exp.
        # Projection chunks (qb0) / deferred qt chunks + the previous qb's
        # tail stages are spread between iterations, after the scores
        # emission so they never delay the ACT-feeding path.
        for kb in range(KB):
            last = kb == KB - 1
            if not (last and qb == QB - 1):
                nqb, nkb = (qb, kb + 1) if not last else (qb + 1, 0)
                attn_next = scores_block(nqb, nkb)
            else:
                attn_next = None
            if qb == 0:
                # kt chunk c feeds scores kb=4c (emitted at kb=4c-2); v
                # chunk c feeds ctx kb=4c (its blocks emitted 4 kb ahead);
                # qt chunk 1 feeds scores(qb1, 0), emitted at kb=31
                if kb % 4 == 2 and kb // 4 + 1 < NCH:
                    project_k(kb // 4 + 1)
                if kb + 4 < S // 128:
                    project_v(kb + 4)
                if kb == 29 and QB > 1:
                    project_q(1)
            else:
                if kb == 16 and qb + 1 < QB:
                    project_q(qb + 1)
                if pending_tail and kb % 2 == 1:
                    pending_tail.pop(0)()
            nc.tensor.matmul(pc0, v_ext[:, kb, 0:65], attn[:, 0:512],
                             start=(kb == 0), stop=last,
                             skip_group_check=True)
            nc.tensor.matmul(pc1, v_ext[:, kb, 65:130], attn[:, 512:1024],
                             start=(kb == 0), stop=last,
                             skip_group_check=True)
            attn = attn_next
        while pending_tail:
            pending_tail.pop(0)()

        # ---- tail for this qb ----
        # The PSUM extraction (DVE) is emitted NOW so its WAR edges land
        # before the next qb's first ctx matmul reuses pc0/pc1 (bufs=1).
        den_row = sp.tile([33, 512], F16, tag="den_row", bufs=2,
                          name="den_row")
        nc.vector.tensor_copy(den_row[0:1, :], pc0[64:65, :])
        nc.vector.tensor_copy(den_row[32:33, :], pc1[64:65, :])
        ctxn = sp.tile([128, 512], BF16, tag="ctxn", bufs=2, name="ctxn")
        nc.vector.tensor_copy(ctxn[0:64, :], pc0[0:64, :])
        nc.vector.tensor_copy(ctxn[64:128, :], pc1[0:64, :])

        def make_stages(qb=qb, den_row=den_row, ctxn=ctxn):
            state = {}
            # the last qb's tail runs in the drain with no scores matmuls
            # left: use the freed scores banks to avoid PSUM WAR stalls
            ptag = "ps" if qb == QB - 1 else "tail"

            def stage_rcp(i):
                # den -> [q-partition, head] via a tiny PE matmul against
                # the selector; rcp then runs on all 128 lanes
                tr = psum.tile([128, 2], F32, tag=ptag, bufs=2, name="tr")
                nc.tensor.matmul(tr, den_row[:, i * 128:(i + 1) * 128], sel)
                rcp = sp.tile([128, 2], F32, tag="rcp", bufs=4, name="rcp")
                nc.vector.reciprocal(rcp, tr)
                state[i] = rcp

            def stage_proj(i):
                rcp = state[i]
                cch = slice(i * 128, (i + 1) * 128)
                po0 = psum.tile([128, 512], F32, tag=ptag, bufs=2,
                                name="po0")
                nc.tensor.matmul(po0, ctxn[0:64, cch], wo_sb[0:64, :])
                po1 = psum.tile([128, 512], F32, tag=ptag, bufs=2,
                                name="po1")
                nc.tensor.matmul(po1, ctxn[64:128, cch], wo_sb[64:128, :])
                tmp = sp.tile([128, 512], F32, tag="tmp", bufs=2, name="tmp")
                nc.vector.tensor_scalar_mul(tmp, po1, rcp[:, 1:2])
                ob = sp.tile([128, 512], F32, tag="ob", bufs=3, name="ob")
                nc.vector.scalar_tensor_tensor(
                    ob, po0, rcp[:, 0:1], tmp, MULT, ADD)
                sb = qb * 4 + i
                nc.sync.dma_start(
                    d["out"].ap()[sb * 128:(sb + 1) * 128, :], ob)

            stages = []
            for i in range(4):
                stages.append(lambda i=i: stage_rcp(i))
            for i in range(4):
                stages.append(lambda i=i: stage_proj(i))
            return stages

        pending_tail = make_stages()

    while pending_tail:
        pending_tail.pop(0)()


def build_program(S=4096, use_mask=False, use_bq=False, use_bk=False,
                  use_bv=False, enable_asserts=False):
    nc = bacc.Bacc("TRN2", target_bir_lowering=False, debug=False,
                   enable_asserts=enable_asserts, num_devices=N_CORES,
                   name="mha")
    d = {
        "xb": nc.dram_tensor("xb", [D_MODEL, S], BF16, kind="ExternalInput"),
        "wq": nc.dram_tensor("wq", [D_MODEL, DL], BF16, kind="ExternalInput"),
        "wk": nc.dram_tensor("wk", [D_MODEL, DL], BF16, kind="ExternalInput"),
        "wv": nc.dram_tensor("wv", [D_MODEL, DL], BF16, kind="ExternalInput"),
        "wo": nc.dram_tensor("wo", [DL, D_MODEL], BF16, kind="ExternalInput"),
        "out": nc.dram_tensor("out", [S, D_MODEL], F32, kind="ExternalOutput"),
    }
    if use_bq:
        d["bq"] = nc.dram_tensor("bq", [DL], F32, kind="ExternalInput")
    if use_bk:
        d["bk"] = nc.dram_tensor("bk", [DL], F32, kind="ExternalInput")
    if use_bv:
        d["bv"] = nc.dram_tensor("bv", [DL], F32, kind="ExternalInput")
    if use_mask:
        d["mb"] = nc.dram_tensor("mb", [128, S // 128], F32,
                                 kind="ExternalInput")
    with tile.TileContext(nc) as tc:
        with ExitStack() as ctx:
            build_kernel(ctx, tc, S, use_mask, use_bq, use_bk, use_bv, d)
    nc.compile()
    return nc


_cache = {}


def _program(key):
    if key not in _cache:
        _cache[key] = build_program(
            S=4096, use_mask=key[0], use_bq=key[1], use_bk=key[2],
            use_bv=key[3])
    return _cache[key]


def kernel(x, mask, Wq, bq, Wk, bk, Wv, bv, Wo, bo, _results_hook=None):
    x = np.asarray(x, np.float32)
    mask = np.asarray(mask)
    B, S, _ = x.shape
    use_mask = bool((mask == 0).any())
    use_bq = bool(np.asarray(bq).any())
    use_bk = bool(np.asarray(bk).any())
    use_bv = bool(np.asarray(bv).any())
    nc = _program((use_mask, use_bq, use_bk, use_bv))

    in_maps = []
    for c in range(N_CORES):
        b, j = divmod(c, N_CORES // B)
        ds = slice(j * DL, (j + 1) * DL)
        m = {
            "xb": np.ascontiguousarray(x[b].T).astype(ml_dtypes.bfloat16),
            "wq": np.ascontiguousarray(Wq[:, ds]).astype(ml_dtypes.bfloat16),
            "wk": np.ascontiguousarray(Wk[:, ds]).astype(ml_dtypes.bfloat16),
            "wv": np.ascontiguousarray(Wv[:, ds]).astype(ml_dtypes.bfloat16),
            "wo": np.ascontiguousarray(Wo[ds, :]).astype(ml_dtypes.bfloat16),
        }
        if use_bq:
            m["bq"] = np.ascontiguousarray(bq[ds], dtype=np.float32)
        if use_bk:
            m["bk"] = np.ascontiguousarray(bk[ds], dtype=np.float32)
        if use_bv:
            m["bv"] = np.ascontiguousarray(bv[ds], dtype=np.float32)
        if use_mask:
            mb = np.where(np.asarray(mask[b]) == 0, -1e9, 0.0).astype(np.float32)
            m["mb"] = np.ascontiguousarray(mb.reshape(S // 128, 128).T)
        in_maps.append(m)

    res = run_bass_kernel_spmd(nc, in_maps, core_ids=list(range(N_CORES)))
    if _results_hook is not None:
        _results_hook(res)
    out = np.zeros((B, S, D_MODEL), np.float32)
    for c in range(N_CORES):
        b = c // (N_CORES // B)
        out[b] += res.results[c]["out"]
    out += np.asarray(bo, np.float32)
    return out


# revision 20
# speedup vs baseline: 1.2405x; 1.0450x over previous
"""Multi-head self-attention (B=2, S=4096, D=512, H=8, Dk=64) on 8 TRN2 cores.

Sharding: data-parallel over batch x head-parallel. Core c handles batch
c//4 and head pair (2*(c%4), 2*(c%4)+1). Each core computes Q/K/V
projections for its 128 model dims, full attention for its two heads, and
a partial output projection against its 128 rows of Wo. The host sums the
four partial outputs per batch and adds bo.

The kernel is paced by the ACT engine (exp of 2*S^2 = 33.5M scores per
core at 1 elem/lane/cycle @ 1.2 GHz ~= 284us); everything else is
structured to keep ACT streaming back-to-back:
  - x arrives host-transposed (xT [512, S] bf16) and is DMA'd in S-chunks;
    Q/K/V projections run per-chunk so the first exp issues ~8us in
    (no xbar DMA transposes, no serial 48us prologue).
  - softmax denominator rides as a 65th column of V (ones), so the ctx
    matmul pair computes ctx+den with no separate den matmuls.
  - den rows are PE-transposed to [q-partition, head] form; reciprocal and
    the output normalization are per-partition DVE ops (no fp32r broadcast
    matmuls, no single-lane 512-wide reciprocals).
  - output projection runs per head (K=64 row-packed concurrent pair) on
    unnormalized ctx in bf16; DVE applies the two reciprocals and sums.
"""

import numpy as np
import ml_dtypes
from contextlib import ExitStack

import concourse.bass as bass
import concourse.tile as tile
from concourse import bacc, mybir
from concourse.bass_utils import run_bass_kernel_spmd

F32 = mybir.dt.float32
F16 = mybir.dt.float16
BF16 = mybir.dt.bfloat16
EXP = mybir.ActivationFunctionType.Exp
MULT = mybir.AluOpType.mult
ADD = mybir.AluOpType.add

D_MODEL = 512
N_HEADS = 8
D_K = 64
N_CORES = 8
DL = 128          # local model dims per core (2 heads)
Q_BLK = 512       # query block (free dim of scores matmuls)
SCALE = 1.0 / np.sqrt(D_K).item()


def build_kernel(ctx, tc, S, use_mask, use_bq, use_bk, use_bv, d):
    nc = tc.nc
    SB = S // 128    # 128-wide s blocks
    QB = S // Q_BLK  # query blocks
    KB = SB          # key blocks of 128
    NCH = S // 512   # s-chunks for the load/projection pipeline

    sp = ctx.enter_context(tc.tile_pool(name="sp", bufs=1))
    psum = ctx.enter_context(tc.tile_pool(name="psum", bufs=1, space="PSUM"))
    # psum budget (8 banks): scores 2x[128,1024]=4, pc_h0/pc_h1 [65,512]=2,
    # tail/proj rotating tag = 2.

    # ---- constants ----
    # selector: den_row[0]=den_h0, den_row[32]=den_h1; tr = den_row_chunk^T
    # @ sel puts den_h0 in tr col 0 and den_h1 in tr col 1. fp16 keeps the
    # matmul single-pass (fp32 matmuls run LOW/HIGH double passes) at
    # ~5e-4 relative rounding on the denominator.
    sel = sp.tile([33, 2], F16, tag="sel")
    nc.vector.memset(sel, 0.0)
    nc.vector.memset(sel[0:1, 0:1], 1.0)
    nc.vector.memset(sel[32:33, 1:2], 1.0)

    # ---- PE warm-up: ~3.4us of matmul activity lifts the HAM clock gate
    # from 1.2 to 2.4 GHz while the first DMAs are in flight ----
    warm = sp.tile([128, 128], BF16, tag="warm")
    nc.vector.memset(warm, 0.0)
    for _ in range(30):
        pw = psum.tile([128, 128], F32, tag="tail", bufs=2, name="pw")
        nc.tensor.matmul(pw, warm, warm)

    # ---- DMAs in consumption order: x chunk 0 first (gates everything),
    # wo last (first needed ~45us in). No DMA transposes -> no xbar
    # hazard. ----
    xt = sp.tile([128, 4, S], BF16, tag="xt")
    xb_ap = d["xb"].ap().rearrange("(t p) s -> p t s", p=128)

    def dma_chunk(c):
        cs = slice(c * 512, (c + 1) * 512)
        nc.sync.dma_start(xt[:, :, cs], xb_ap[:, :, cs])

    dma_chunk(0)
    wq_sb = sp.tile([128, 4, 128], BF16, tag="wq")
    nc.sync.dma_start(wq_sb, d["wq"].ap().rearrange("(t p) d -> p t d", p=128))
    wk_sb = sp.tile([128, 4, 128], BF16, tag="wk")
    nc.sync.dma_start(wk_sb, d["wk"].ap().rearrange("(t p) d -> p t d", p=128))
    wv_sb = sp.tile([128, 4, 128], BF16, tag="wv")
    nc.sync.dma_start(wv_sb, d["wv"].ap().rearrange("(t p) d -> p t d", p=128))
    dma_chunk(1)
    if use_bq:
        bq_sb = sp.tile([128, 1], F32, tag="bq")
        nc.sync.dma_start(bq_sb, d["bq"].ap()[:, None])
    if use_bk:
        bk_sb = sp.tile([128, 1], F32, tag="bk")
        nc.sync.dma_start(bk_sb, d["bk"].ap()[:, None])
    if use_bv:
        bv_sb = sp.tile([1, 128], F32, tag="bv")
        nc.sync.dma_start(bv_sb, d["bv"].ap()[None, :])
        ones_row = sp.tile([1, 128], F32, tag="ones_row")
        nc.vector.memset(ones_row, 1.0)
    if use_mask:
        mb_sb = sp.tile([128, KB], F32, tag="mb")
        nc.sync.dma_start(mb_sb, d["mb"].ap())
    for c in range(2, NCH):
        dma_chunk(c)
    wo_sb = sp.tile([128, 512], BF16, tag="wo")
    nc.sync.dma_start(wo_sb, d["wo"].ap())

    qt = sp.tile([128, S], BF16, tag="qt")
    kt = sp.tile([128, S], BF16, tag="kt")
    # v_ext: per key block, cols 0:64 = V head0, 64 = ones, 65:129 = V
    # head1, 129 = ones -> the ctx matmul's 65th output row is the softmax
    # denominator.
    v_ext = sp.tile([128, SB, 130], BF16, tag="v")
    nc.vector.memset(v_ext[:, :, 64:65], 1.0)
    nc.vector.memset(v_ext[:, :, 129:130], 1.0)

    def project_qk(c, dst, w_sb, b_sb, ptag):
        cs = slice(c * 512, (c + 1) * 512)
        pp = psum.tile([128, 512], F32, tag=ptag, bufs=2, name="pp")
        for t in range(4):
            nc.tensor.matmul(pp, w_sb[:, t, :], xt[:, t, cs],
                             start=(t == 0), stop=(t == 3))
        if b_sb is not None:
            nc.vector.tensor_scalar_add(dst[:, cs], pp, b_sb[:, 0:1])
        else:
            nc.vector.tensor_copy(dst[:, cs], pp)

    def project_v(sb, ptag):
        pv = psum.tile([128, 128], F32, tag=ptag, bufs=2, name="pv")
        for t in range(4):
            nc.tensor.matmul(
                pv, xt[:, t, sb * 128:(sb + 1) * 128], wv_sb[:, t, :],
                start=(t == 0), stop=(t == 3 and not use_bv))
        if use_bv:
            nc.tensor.matmul(pv, ones_row[0:1, :], bv_sb[0:1, :],
                             start=False, stop=True)
        nc.vector.tensor_copy(v_ext[:, sb, 0:64], pv[:, 0:64])
        nc.vector.tensor_copy(v_ext[:, sb, 65:129], pv[:, 64:128])

    def project_q(c, ptag="tail"):
        project_qk(c, qt, wq_sb, bq_sb if use_bq else None, ptag)

    def project_k(c, ptag="tail"):
        project_qk(c, kt, wk_sb, bk_sb if use_bk else None, ptag)

    # prologue: chunk 0 (+ qt chunk 1 for the warmup interleave).
    project_q(0)
    project_k(0)
    project_v(0, "tail")
    project_q(1)

    # ---- attention: flat (qb, kb) sequence ----
    # Warmup interleave: qb0 and qb1 alternate in 4-kb strides, so the
    # feasible exp work per projected x-chunk doubles (qb1 reuses the
    # same kt/v) and ACT streams while projections catch up. qb1's
    # accumulators borrow the "tail" psum buffers (no tail work exists
    # during warmup); projections during warmup borrow the "ps" buffers.
    seq = []
    for step in range(2 * (KB // 4)):
        q = step % 2
        k0 = (step // 2) * 4
        seq += [(q, k0 + j) for j in range(4)]
    for qb in range(2, QB):
        seq += [(qb, kb) for kb in range(KB)]
    assert len(seq) == QB * KB and len(set(seq)) == QB * KB

    # just-in-time projection emission: unit index -> closures
    emit = {}

    def emit_at(u, f):
        emit.setdefault(u, []).append(f)

    for c in range(1, NCH):  # kt chunk c first used at unit 8c (qb0 kb=4c)
        emit_at(max(0, 8 * c - 6), lambda c=c: project_k(c, "ps"))
    for sb in range(1, SB):  # v block sb first used at unit 8*(sb//4)+sb%4
        emit_at(max(0, 8 * (sb // 4) + sb % 4 - 6),
                lambda sb=sb: project_v(sb, "ps"))
    if QB > 2:
        emit_at(52, lambda: project_q(2, "ps"))
    for qq in range(3, QB):  # needed at unit 64+(qq-2)*32
        emit_at(64 + (qq - 3) * 32 + 16, lambda qq=qq: project_q(qq))

    def scores_block(qb, kb):
        qs = slice(qb * Q_BLK, (qb + 1) * Q_BLK)
        ks = slice(kb * 128, (kb + 1) * 128)
        ps = psum.tile([128, 1024], F32, tag="ps", bufs=2, name="ps")
        nc.tensor.matmul(ps[:, 0:512], kt[0:64, ks], qt[0:64, qs])
        nc.tensor.matmul(ps[:, 512:1024], kt[64:128, ks], qt[64:128, qs])
        attn = sp.tile([128, 1024], BF16, tag="attn", bufs=4, name="attn")
        nc.scalar.activation(
            attn, ps, EXP, scale=SCALE,
            bias=mb_sb[:, kb:kb + 1] if use_mask else 0.0)
        return attn

    def extract_tail(qb, pc0, pc1):
        # PSUM extraction (DVE), emitted at the qb's last ctx matmul so
        # its WAR edges land before pc0/pc1 are reused.
        den_row = sp.tile([33, 512], F16, tag="den_row", bufs=2,
                          name="den_row")
        nc.vector.tensor_copy(den_row[0:1, :], pc0[64:65, :])
        nc.vector.tensor_copy(den_row[32:33, :], pc1[64:65, :])
        ctxn = sp.tile([128, 512], BF16, tag="ctxn", bufs=2, name="ctxn")
        nc.vector.tensor_copy(ctxn[0:64, :], pc0[0:64, :])
        nc.vector.tensor_copy(ctxn[64:128, :], pc1[0:64, :])

        state = {}
        final = qb == QB - 1

        def stage_rcp(i):
            # den -> [q-partition, head] via a tiny PE matmul against the
            # selector; rcp then runs on all 128 lanes
            tr = psum.tile([128, 2], F32, tag="ps" if final else "tail",
                           bufs=2, name="tr")
            nc.tensor.matmul(tr, den_row[:, i * 128:(i + 1) * 128], sel)
            rcp = sp.tile([128, 2], F32, tag="rcp", bufs=4, name="rcp")
            nc.vector.reciprocal(rcp, tr)
            state[i] = rcp

        def stage_proj(i):
            rcp = state[i]
            cch = slice(i * 128, (i + 1) * 128)
            if final:
                # spread the drain's po allocations across four psum tag
                # pools (all free by now) so chunks pipeline instead of
                # serializing on 2-buffer WAR
                t0, b0 = ("ps", 2) if i % 2 == 0 else ("tail", 2)
                t1, b1 = ("pc0", 1) if i % 2 == 0 else ("pc1", 1)
            else:
                t0, b0 = t1, b1 = ("tail", 2)
            po0 = psum.tile([128, 512], F32, tag=t0, bufs=b0, name="po0")
            nc.tensor.matmul(po0, ctxn[0:64, cch], wo_sb[0:64, :])
            po1 = psum.tile([128, 512], F32, tag=t1, bufs=b1, name="po1")
            nc.tensor.matmul(po1, ctxn[64:128, cch], wo_sb[64:128, :])
            tmp = sp.tile([128, 512], BF16, tag="tmp", bufs=2, name="tmp")
            nc.vector.tensor_scalar_mul(tmp, po1, rcp[:, 1:2])
            ob = sp.tile([128, 512], F32, tag="ob", bufs=3, name="ob")
            nc.vector.scalar_tensor_tensor(
                ob, po0, rcp[:, 0:1], tmp, MULT, ADD)
            sb = qb * 4 + i
            nc.sync.dma_start(
                d["out"].ap()[sb * 128:(sb + 1) * 128, :], ob)

        stages = []
        for i in range(4):
            stages.append(lambda i=i: stage_rcp(i))
        for i in range(4):
            stages.append(lambda i=i: stage_proj(i))
        return stages

    # scores run 2 units ahead of ctx (matches the 2-deep ps rotation and
    # keeps ACT fed across qb boundaries)
    tails = []
    pcs = {}
    attn_q = [scores_block(*seq[0]), scores_block(*seq[1])]
    for i, (qb, kb) in enumerate(seq):
        if i + 2 < len(seq):
            attn_q.append(scores_block(*seq[i + 2]))
        for f in emit.get(i, []):
            f()
        if qb >= 2 and kb % 2 == 1 and tails:
            tails.pop(0)()
        if kb == 0:
            if qb == 1:
                pcs[qb] = (
                    psum.tile([65, 512], F32, tag="tail", bufs=2,
                              name="pc0b"),
                    psum.tile([65, 512], F32, tag="tail", bufs=2,
                              name="pc1b"))
            else:
                pcs[qb] = (
                    psum.tile([65, 512], F32, tag="pc0", bufs=1,
                              name="pc0"),
                    psum.tile([65, 512], F32, tag="pc1", bufs=1,
                              name="pc1"))
        pc0, pc1 = pcs[qb]
        attn = attn_q.pop(0)
        nc.tensor.matmul(pc0, v_ext[:, kb, 0:65], attn[:, 0:512],
                         start=(kb == 0), stop=(kb == KB - 1),
                         skip_group_check=True)
        nc.tensor.matmul(pc1, v_ext[:, kb, 65:130], attn[:, 512:1024],
                         start=(kb == 0), stop=(kb == KB - 1),
                         skip_group_check=True)
        if kb == KB - 1:
            tails.extend(extract_tail(qb, pc0, pc1))

    while tails:
        tails.pop(0)()


def build_program(S=4096, use_mask=False, use_bq=False, use_bk=False,
                  use_bv=False, enable_asserts=False):
    nc = bacc.Bacc("TRN2", target_bir_lowering=False, debug=False,
                   enable_asserts=enable_asserts, num_devices=N_CORES,
                   name="mha")
    d = {
        "xb": nc.dram_tensor("xb", [D_MODEL, S], BF16, kind="ExternalInput"),
        "wq": nc.dram_tensor("wq", [D_MODEL, DL], BF16, kind="ExternalInput"),
        "wk": nc.dram_tensor("wk", [D_MODEL, DL], BF16, kind="ExternalInput"),
        "wv": nc.dram_tensor("wv", [D_MODEL, DL], BF16, kind="ExternalInput"),
        "wo": nc.dram_tensor("wo", [DL, D_MODEL], BF16, kind="ExternalInput"),
        "out": nc.dram_tensor("out", [S, D_MODEL], F32, kind="ExternalOutput"),
    }
    if use_bq:
        d["bq"] = nc.dram_tensor("bq", [DL], F32, kind="ExternalInput")
    if use_bk:
        d["bk"] = nc.dram_tensor("bk", [DL], F32, kind="ExternalInput")
    if use_bv:
        d["bv"] = nc.dram_tensor("bv", [DL], F32, kind="ExternalInput")
    if use_mask:
        d["mb"] = nc.dram_tensor("mb", [128, S // 128], F32,
                                 kind="ExternalInput")
    with tile.TileContext(nc) as tc:
        with ExitStack() as ctx:
            build_kernel(ctx, tc, S, use_mask, use_bq, use_bk, use_bv, d)
    nc.compile()
    return nc


_cache = {}


def _program(key):
    if key not in _cache:
        _cache[key] = build_program(
            S=4096, use_mask=key[0], use_bq=key[1], use_bk=key[2],
            use_bv=key[3])
    return _cache[key]


def kernel(x, mask, Wq, bq, Wk, bk, Wv, bv, Wo, bo, _results_hook=None):
    x = np.asarray(x, np.float32)
    mask = np.asarray(mask)
    B, S, _ = x.shape
    use_mask = bool((mask == 0).any())
    use_bq = bool(np.asarray(bq).any())
    use_bk = bool(np.asarray(bk).any())
    use_bv = bool(np.asarray(bv).any())
    nc = _program((use_mask, use_bq, use_bk, use_bv))

    in_maps = []
    for c in range(N_CORES):
        b, j = divmod(c, N_CORES // B)
        ds = slice(j * DL, (j + 1) * DL)
        m = {
            "xb": np.ascontiguousarray(x[b].T).astype(ml_dtypes.bfloat16),
            "wq": np.ascontiguousarray(Wq[:, ds]).astype(ml_dtypes.bfloat16),
            "wk": np.ascontiguousarray(Wk[:, ds]).astype(ml_dtypes.bfloat16),
            "wv": np.ascontiguousarray(Wv[:, ds]).astype(ml_dtypes.bfloat16),
            "wo": np.ascontiguousarray(Wo[ds, :]).astype(ml_dtypes.bfloat16),
        }
        if use_bq:
            m["bq"] = np.ascontiguousarray(bq[ds], dtype=np.float32)
        if use_bk:
            m["bk"] = np.ascontiguousarray(bk[ds], dtype=np.float32)
        if use_bv:
            m["bv"] = np.ascontiguousarray(bv[ds], dtype=np.float32)
        if use_mask:
            mb = np.where(np.asarray(mask[b]) == 0, -1e9, 0.0).astype(np.float32)
            m["mb"] = np.ascontiguousarray(mb.reshape(S // 128, 128).T)
        in_maps.append(m)

    res = run_bass_kernel_spmd(nc, in_maps, core_ids=list(range(N_CORES)))
    if _results_hook is not None:
        _results_hook(res)
    out = np.zeros((B, S, D_MODEL), np.float32)
    for c in range(N_CORES):
        b = c // (N_CORES // B)
        out[b] += res.results[c]["out"]
    out += np.asarray(bo, np.float32)
    return out
